# revision 14
# baseline (speedup 1.0000x reference)
"""Trainium2 Bass kernel for nn_Actor_decoder (8-core data-parallel over batch).

Reference computation (B=64, N=20, K=5, FEAT=320, MID=128, M=C(20,5)//2=7752):
    h = MLP(candidate_feature)                      # [B, N, MID], leaky_relu x2
    multi = S @ h (per batch)                        # S: [M, N] rows avg 5 of 20
    score = relu(multi @ Wo) @ u                     # [B, M]
    out = softmax(10 * tanh(score), axis=1)

Kernel strategy (per core, 8 batches):
  - Reassociate: multi @ Wo = S @ (h @ Wo). Binary S (exact in bf16); 1/5
    folded into the one-hot u columns.
  - MLP in fp32, dims-on-partitions; bias + parametric-relu fused into one
    ScalarE activation per layer (Prelu, not Lrelu: parametric_relu shares the
    exp_and_others spline table with Relu/Tanh/Exp/Copy, so the whole kernel
    needs exactly one ACT table load).
  - Single fused z+u stream over 32 (half, chunk) blocks: per block, four
    K=20 S-matmuls (row-group packed, 4-way concurrent) write two 2-bank PSUM
    supertiles; each supertile is evacuated (relu, fp32->bf16) by ScalarE or
    VectorE, greedily assigned to whichever engine's queue is shorter
    (ScalarE is ~10% faster per element, so it takes more tiles). The u-dot
    accumulating quads (K=128, one-hot u/5 columns, 4 col-groups concurrent)
    for chunk c-1 are interleaved one chunk behind the evacs, so the whole
    u-phase hides inside the evac-bound stream.
  - Chunk 15 is narrow: 72 real m-columns instead of 512 (no padded evac
    work). Score rows r>=16 and the 440 never-written columns of row 15 stay
    exactly 0 through the accumulation, contributing exp(0)=1 each to the
    softmax row-sums; a single [128,128] matmul computes the per-batch
    denominator with the -440 correction folded in AND broadcasts it to all
    partitions in one shot.
  - Per-half softmax: half 0's tanh/exp/recip/mul/output-DMAs are staged one
    op per chunk-slot into the first chunks of half 1's stream, fully hidden.
    Output DMAs are consolidated: one 4-batch main-block DMA + one 4-batch
    remainder DMA per half.
"""

import numpy as np
import ml_dtypes
from itertools import combinations

import concourse.bass as bass
import concourse.mybir as mybir
import concourse.tile as tile
from concourse import bacc
from concourse.bass_utils import run_bass_kernel_spmd

F32 = mybir.dt.float32
BF16 = mybir.dt.bfloat16
AF = mybir.ActivationFunctionType

B, N, KC, FEAT, MID = 64, 20, 5, 320, 128
M = 7752                      # C(20,5) // 2
N_CORES = 8
BPC = B // N_CORES            # batches per core
NCHUNK = 16                   # m-chunks: 15 x 512 + 1 x 72
MREM = M - 15 * 512           # 72
PAD_ONES = 512 - MREM         # 440 implicit exp(0)=1 columns in row 15
NEG_SLOPE = 0.01


def _build_nc():
    nc = bacc.Bacc()
    xT = nc.declare_dram_parameter("xT", [384, 160], F32, isOutput=False)
    w0 = nc.declare_dram_parameter("w0", [384, 128], F32, isOutput=False)
    w1 = nc.declare_dram_parameter("w1", [128, 128], F32, isOutput=False)
    w2 = nc.declare_dram_parameter("w2", [128, 128], F32, isOutput=False)
    wo = nc.declare_dram_parameter("wo", [128, 128], F32, isOutput=False)
    b0 = nc.declare_dram_parameter("b0", [128], F32, isOutput=False)
    b1 = nc.declare_dram_parameter("b1", [128], F32, isOutput=False)
    b2 = nc.declare_dram_parameter("b2", [128], F32, isOutput=False)
    st4 = nc.declare_dram_parameter("st4", [128, M], BF16, isOutput=False)
    uc = nc.declare_dram_parameter("uc", [128, 512], BF16, isOutput=False)
    b4w = nc.declare_dram_parameter("b4w", [128, 128], F32, isOutput=False)
    out = nc.declare_dram_parameter("out", [BPC, M], F32, isOutput=True)

    CW = [512] * 15 + [MREM]

    with tile.TileContext(nc) as tc:
        with (
            tc.tile_pool(name="persist", bufs=1) as persist,
            tc.tile_pool(name="mlp_sb", bufs=2) as mlp_sb,
            tc.tile_pool(name="zs_sb", bufs=33) as zs_sb,
            tc.tile_pool(name="soft_sb", bufs=2) as soft_sb,
        ):
            # ---- constant / weight loads ----
            # MLP-critical weights on the SP queue; everything else on gpsimd
            # (SWDGE) so its issue cost stays off the queues the MLP's latency
            # chain runs through.
            xTt = persist.tile([128, 3, 160], F32, tag="xTt")
            nc.sync.dma_start(out=xTt[:], in_=xT[:].rearrange("(c p) f -> p c f", p=128))
            w0t = persist.tile([128, 3, 128], F32, tag="w0t")
            nc.sync.dma_start(out=w0t[:], in_=w0[:].rearrange("(c p) f -> p c f", p=128))
            w1t = persist.tile([128, 128], F32, tag="w1t")
            nc.sync.dma_start(out=w1t[:], in_=w1[:])
            w2t = persist.tile([128, 128], F32, tag="w2t")
            nc.sync.dma_start(out=w2t[:], in_=w2[:])
            wot = persist.tile([128, 128], F32, tag="wot")
            nc.sync.dma_start(out=wot[:], in_=wo[:])
            b0t = persist.tile([128, 1], F32, tag="b0t")
            nc.gpsimd.dma_start(out=b0t[:], in_=b0[:].rearrange("(p o) -> p o", o=1))
            b1t = persist.tile([128, 1], F32, tag="b1t")
            nc.gpsimd.dma_start(out=b1t[:], in_=b1[:].rearrange("(p o) -> p o", o=1))
            b2t = persist.tile([128, 1], F32, tag="b2t")
            nc.gpsimd.dma_start(out=b2t[:], in_=b2[:].rearrange("(p o) -> p o", o=1))
            st4t = persist.tile([128, M], BF16, tag="st4t")
            for lo, hi in ((0, 2048), (2048, 4096), (4096, 6144), (6144, M)):
                nc.gpsimd.dma_start(out=st4t[:, lo:hi], in_=st4[:, lo:hi])
            uct = persist.tile([128, 512], BF16, tag="uct")
            nc.gpsimd.dma_start(out=uct[:], in_=uc[:])
            b4wt = persist.tile([128, 128], F32, tag="b4wt")
            nc.gpsimd.dma_start(out=b4wt[:], in_=b4w[:])

            # ---- ACT spline-table preload: tiny ops on a memset tile force
            # the (single) table set to load while input DMAs are in flight.
            pre = mlp_sb.tile([128, 1], F32, tag="pre", name="pre")
            nc.vector.memset(pre[:], 0.25)
            pre2 = mlp_sb.tile([128, 1], F32, tag="pre2", name="pre2")
            nc.scalar.activation(out=pre2[:], in_=pre[:], func=AF.Prelu,
                                 bias=0.0, scale=1.0, alpha=NEG_SLOPE)
            nc.scalar.activation(out=pre2[:], in_=pre[:], func=AF.Tanh)
            nc.scalar.activation(out=pre2[:], in_=pre[:], func=AF.Exp)
            nc.scalar.copy(out=pre2[:], in_=pre[:])

            # ---- MLP (own psum pool, released before the z stream) ----
            rows = []
            with tc.tile_pool(name="mlp_ps", bufs=2, space="PSUM") as mlp_ps:
                h1ps = mlp_ps.tile([128, 160], F32, tag="mp", name="h1ps")
                for k in range(3):
                    nc.tensor.matmul(h1ps[:], w0t[:, k, :], xTt[:, k, :],
                                     start=(k == 0), stop=(k == 2))
                h1 = mlp_sb.tile([128, 160], F32, tag="h1")
                nc.scalar.activation(out=h1[:], in_=h1ps[:], func=AF.Prelu,
                                     bias=b0t[:], scale=1.0, alpha=NEG_SLOPE)
                h2ps = mlp_ps.tile([128, 160], F32, tag="mp", name="h2ps")
                nc.tensor.matmul(h2ps[:], w1t[:], h1[:], start=True, stop=True)
                h2 = mlp_sb.tile([128, 160], F32, tag="h2")
                nc.scalar.activation(out=h2[:], in_=h2ps[:], func=AF.Prelu,
                                     bias=b1t[:], scale=1.0, alpha=NEG_SLOPE)
                h3ps = mlp_ps.tile([128, 160], F32, tag="mp", name="h3ps")
                nc.tensor.matmul(h3ps[:], w2t[:], h2[:], start=True, stop=True)
                # h3 written 3x into [128, 8, 60] (per-batch contiguous
                # replicas): the [128, 60] stationary slice h3r[:, b, :] then
                # emits all three replicas of a batch in ONE rows-matmul.
                h3r = mlp_sb.tile([128, 8, 60], F32, tag="h3r", name="h3r")
                for rep in range(3):
                    nc.scalar.activation(
                        out=h3r[:, :, 20 * rep:20 * rep + 20],
                        in_=h3ps[:].rearrange("p (b n) -> p b n", n=20),
                        func=AF.Identity, bias=b2t[:], scale=1.0)
                # rows[(half,p)]: partitions 64s+(20*rep+n) = (h[4h+2p+s] @
                # Wo)[n, :] for s in {0,1}, rep in {0,1,2}; K=64 stationaries
                # for the z S-matmuls keep PE row occupancy ~94% so the clock
                # monitor holds the 2.4 GHz p-state through the stream.
                for half in range(2):
                    for p in range(2):
                        rps = mlp_ps.tile([128, 128], F32, tag="mp",
                                          name=f"rps{half}_{p}")
                        for s_ in range(2):
                            b = half * 4 + 2 * p + s_
                            nc.tensor.matmul(
                                rps[64 * s_:64 * s_ + 60, :],
                                h3r[:, b, :], wot[:],
                                start=True, stop=True,
                                tile_position=(0, 64 * s_))
                        rt = persist.tile([128, 128], BF16,
                                          tag=f"rows{half}_{p}",
                                          name=f"rows{half}_{p}")
                        # pad rows 60-63/124-127 must be exactly 0 (not stale
                        # PSUM bits): NaN x 0 would poison the K=64 matmul.
                        nc.vector.memset(rt[:], 0.0)
                        nc.scalar.copy(out=rt[0:60, :], in_=rps[0:60, :])
                        nc.scalar.copy(out=rt[64:124, :], in_=rps[64:124, :])
                        rows.append(rt)  # index 2*half + p

            # ---- z stream (all 8 PSUM banks: 4-slot ring, no coupling
            # between the PE and the evac engines), then a separate u phase
            # at the warm clock with the softmax staged inside it.
            # greedy evac-engine balancing (measured sustained ns/op)
            eng_t = {"a": 0.0, "d": 0.0}

            def pick(nelem):
                ca = nelem * 0.833 + 180.0
                cd = nelem * 0.917 + 190.0
                if eng_t["a"] + ca <= eng_t["d"] + cd:
                    eng_t["a"] += ca
                    return "a"
                eng_t["d"] += cd
                return "d"

            zsts = {}
            with tc.tile_pool(name="zq_ps", bufs=4, space="PSUM") as zq_ps:

                def emit_chunk(half, c):
                    w = CW[c]
                    zst = zs_sb.tile([128, 4, 512], BF16, tag="zs",
                                     name=f"zst{half}_{c}")
                    for pr in range(2):
                        zps = zq_ps.tile([128, 2, 512], F32, tag="zq",
                                         name=f"zq{half}_{c}_{pr}")
                        for s_ in range(2):
                            nc.tensor.matmul(
                                zps[:, s_, 0:w],
                                rows[2 * half + pr][64 * s_:64 * s_ + 64, :],
                                st4t[64 * s_:64 * s_ + 64,
                                     512 * c:512 * c + w],
                                start=True, stop=True,
                                tile_position=(64 * s_, 0))
                        e = pick(2 * w)
                        src = zps[:, 0:2, 0:w]
                        dst = zst[:, 2 * pr:2 * pr + 2, 0:w]
                        if e == "a":
                            nc.scalar.activation(out=dst, in_=src, func=AF.Relu)
                        else:
                            nc.vector.tensor_scalar_max(out=dst, in0=src,
                                                        scalar1=0.0)
                    zsts[(half, c)] = zst

                for hc in [(h, c) for h in range(2) for c in range(NCHUNK)]:
                    emit_chunk(*hc)

            with tc.tile_pool(name="score_ps", bufs=1, space="PSUM") as score_ps:
                scores = []
                for half in range(2):
                    sct = score_ps.tile([128, 512], F32, tag=f"sc{half}",
                                        name=f"sc{half}")
                    scores.append(sct)

                def emit_u(half, c):
                    w = CW[c]
                    zst = zsts.pop((half, c))
                    for g in range(4):
                        nc.tensor.matmul(
                            scores[half][32 * g:32 * g + 32, 0:w],
                            uct[:, 32 * c:32 * c + 32], zst[:, g, 0:w],
                            start=(c == 0), stop=(c == NCHUNK - 1),
                            tile_position=(0, 32 * g), skip_group_check=True)

                def softmax_stages(half):
                    st = {}

                    def s1():  # tanh
                        st["t"] = soft_sb.tile([128, 512], F32, tag=f"t{half}", name=f"t{half}")
                        nc.scalar.activation(out=st["t"][:], in_=scores[half][:],
                                             func=AF.Tanh)
                        eng_t["a"] += 800.0

                    def s2():  # exp with row-sum accumulator
                        st["e"] = soft_sb.tile([128, 512], F32, tag=f"e{half}", name=f"e{half}")
                        st["rs"] = soft_sb.tile([128, 1], F32, tag=f"rs{half}", name=f"rs{half}")
                        nc.scalar.activation(out=st["e"][:], in_=st["t"][:],
                                             func=AF.Exp, scale=10.0,
                                             accum_out=st["rs"][:])
                        eng_t["a"] += 1000.0

                    def s3():  # denominator matmul (sum+pad-correct+broadcast)
                        st["rbc"] = score_ps.tile([128, 512], F32,
                                                  tag=f"sc{half}",
                                                  name=f"rbc{half}")
                        nc.tensor.matmul(st["rbc"][:, 0:1], b4wt[:],
                                         st["rs"][:], start=True, stop=True,
                                         skip_group_check=True)
                        st["rec"] = soft_sb.tile([128, 1], F32, tag=f"rec{half}", name=f"rec{half}")
                        nc.vector.reciprocal(out=st["rec"][:],
                                             in_=st["rbc"][:, 0:1])
                        eng_t["d"] += 260.0

                    def s4():  # scale + output DMAs
                        probs = soft_sb.tile([128, 512], F32, tag=f"pr{half}", name=f"pr{half}")
                        nc.vector.tensor_scalar_mul(out=probs[:], in0=st["e"][:],
                                                    scalar1=st["rec"][:])
                        eng_t["d"] += 760.0
                        # per-batch main blocks: a DMA source AP may carry only
                        # ONE partition-dim pair, so the four 15-row blocks
                        # cannot be fused into a single (g,r) pattern.
                        for g in range(4):
                            b = 4 * half + g
                            eng = nc.sync if g % 2 == 0 else nc.gpsimd
                            eng.dma_start(
                                out=out[b, 0:15 * 512]
                                    .rearrange("(r j) -> r j", j=512),
                                in_=probs[32 * g:32 * g + 15, :])
                        # remainders consolidated: stride-32 partition dim
                        # (single level) x 72 cols -> 4 DRAM rows.
                        nc.gpsimd.dma_start(
                            out=out[4 * half:4 * half + 4, 15 * 512:M]
                                .rearrange("b (x j) -> b x j", x=1),
                            in_=probs[:].rearrange("(g r) j -> g r j", g=4)
                                [:, 15:16, 0:MREM])

                    return [s1, s2, s3, s4]

                for c in range(NCHUNK):
                    emit_u(0, c)
                post = softmax_stages(0)
                for c in range(NCHUNK):
                    emit_u(1, c)
                    if post:
                        post.pop(0)()
                for s in post:
                    s()
                for s in softmax_stages(1):
                    s()

    nc.compile()
    return nc


_NC_CACHE = None


def _get_nc():
    global _NC_CACHE
    if _NC_CACHE is None:
        _NC_CACHE = _build_nc()
    return _NC_CACHE


def _host_constants(W0, b0, W1, b1, W2, b2, Wo, u):
    # binary S^T replicated at the four 32-row offsets; narrow chunk 15
    combs = list(combinations(range(N), KC))
    rows_idx = np.zeros((M, KC), np.int64)
    for i, cmb in enumerate(combs[:M]):
        rows_idx[i] = cmb
    stT = np.zeros((N, M), np.float32)
    stT[rows_idx.T, np.arange(M)[None, :].repeat(KC, 0)] = 1.0
    # K=64 layout: two 64-row groups, each holding 3 scaled replicas of S^T
    # (scales 0.375+0.375+0.25 = 1 exactly, each exact in bf16) + 4 zero rows.
    st4 = np.zeros((128, M), np.float32)
    for s in range(2):
        for rep, sc in enumerate((0.375, 0.375, 0.25)):
            st4[64 * s + 20 * rep:64 * s + 20 * rep + N, :] = sc * stT
    # one-hot u/5 columns: uc[d, 32c + r] = u[d]/5 if r == c < 16
    ucm = np.zeros((128, 512), np.float32)
    v = (u.astype(np.float64) / KC).astype(np.float32)
    for c in range(NCHUNK):
        ucm[:, 32 * c + c] = v
    # fused denominator weight: rbc[p] = sum_k b4w[k,p] * rs[k] gives, for
    # every partition p of group g, (sum_{c<16} rs[32g+c]) - PAD_ONES
    # (row 32g+16's rs is exactly 512, so -PAD_ONES/512 * rs[32g+16] = -440).
    b4w = np.zeros((128, 128), np.float32)
    for g in range(4):
        b4w[32 * g:32 * g + NCHUNK, 32 * g:32 * g + 32] = 1.0
        b4w[32 * g + NCHUNK, 32 * g:32 * g + 32] = -float(PAD_ONES) / 512.0
    w0p = np.zeros((384, 128), np.float32)
    w0p[:FEAT] = W0
    return {
        "w0": w0p, "w1": np.ascontiguousarray(W1, np.float32),
        "w2": np.ascontiguousarray(W2, np.float32),
        "wo": np.ascontiguousarray(Wo, np.float32),
        "b0": np.ascontiguousarray(b0, np.float32),
        "b1": np.ascontiguousarray(b1, np.float32),
        "b2": np.ascontiguousarray(b2, np.float32),
        "st4": st4.astype(ml_dtypes.bfloat16),
        "uc": ucm.astype(ml_dtypes.bfloat16),
        "b4w": b4w,
    }


def kernel(batch, candidate_feature, W0, b0, W1, b1, W2, b2, Wo, u):
    cf = np.ascontiguousarray(np.asarray(candidate_feature), dtype=np.float32)
    consts = _host_constants(np.asarray(W0), np.asarray(b0), np.asarray(W1),
                             np.asarray(b1), np.asarray(W2), np.asarray(b2),
                             np.asarray(Wo), np.asarray(u))
    in_maps = []
    for core in range(N_CORES):
        xc = cf[core * BPC:(core + 1) * BPC]            # [8, 20, 320]
        xTp = np.zeros((384, 160), np.float32)
        xTp[:FEAT] = xc.reshape(BPC * N, FEAT).T        # col = b*20 + n
        in_maps.append({"xT": xTp, **consts})
    nc = _get_nc()
    res = run_bass_kernel_spmd(nc, in_maps, core_ids=list(range(N_CORES)))
    return np.concatenate([res.results[i]["out"] for i in range(N_CORES)], axis=0)


# revision 17
# speedup vs baseline: 1.0645x; 1.0645x over previous
"""Trainium2 Bass kernel for nn_Actor_decoder (8-core data-parallel over batch).

Reference computation (B=64, N=20, K=5, FEAT=320, MID=128, M=C(20,5)//2=7752):
    h = MLP(candidate_feature)                      # [B, N, MID], leaky_relu x2
    multi = S @ h (per batch)                        # S: [M, N] rows avg 5 of 20
    score = relu(multi @ Wo) @ u                     # [B, M]
    out = softmax(10 * tanh(score), axis=1)

Kernel strategy (per core, 8 batches):
  - Reassociate: multi @ Wo = S @ (h @ Wo). Binary S (exact in bf16); 1/5
    folded into the one-hot u columns.
  - MLP in fp32, dims-on-partitions; bias + parametric-relu fused into one
    ScalarE activation per layer (Prelu, not Lrelu: parametric_relu shares the
    exp_and_others spline table with Relu/Tanh/Exp/Copy, so the whole kernel
    needs exactly one ACT table load).
  - Single fused z+u stream over 32 (half, chunk) blocks: per block, four
    K=20 S-matmuls (row-group packed, 4-way concurrent) write two 2-bank PSUM
    supertiles; each supertile is evacuated (relu, fp32->bf16) by ScalarE or
    VectorE, greedily assigned to whichever engine's queue is shorter
    (ScalarE is ~10% faster per element, so it takes more tiles). The u-dot
    accumulating quads (K=128, one-hot u/5 columns, 4 col-groups concurrent)
    for chunk c-1 are interleaved one chunk behind the evacs, so the whole
    u-phase hides inside the evac-bound stream.
  - Chunk 15 is narrow: 72 real m-columns instead of 512 (no padded evac
    work). Score rows r>=16 and the 440 never-written columns of row 15 stay
    exactly 0 through the accumulation, contributing exp(0)=1 each to the
    softmax row-sums; a single [128,128] matmul computes the per-batch
    denominator with the -440 correction folded in AND broadcasts it to all
    partitions in one shot.
  - Per-half softmax: half 0's tanh/exp/recip/mul/output-DMAs are staged one
    op per chunk-slot into the first chunks of half 1's stream, fully hidden.
    Output DMAs are consolidated: one 4-batch main-block DMA + one 4-batch
    remainder DMA per half.
"""

import numpy as np
import ml_dtypes
from itertools import combinations

import concourse.bass as bass
import concourse.mybir as mybir
import concourse.tile as tile
from concourse import bacc
from concourse.bass_utils import run_bass_kernel_spmd

F32 = mybir.dt.float32
BF16 = mybir.dt.bfloat16
AF = mybir.ActivationFunctionType

B, N, KC, FEAT, MID = 64, 20, 5, 320, 128
M = 7752                      # C(20,5) // 2
N_CORES = 8
BPC = B // N_CORES            # batches per core
NCHUNK = 16                   # m-chunks: 15 x 512 + 1 x 72
MREM = M - 15 * 512           # 72
PAD_ONES = 512 - MREM         # 440 implicit exp(0)=1 columns in row 15
NEG_SLOPE = 0.01


def _build_nc():
    nc = bacc.Bacc()
    xT = nc.declare_dram_parameter("xT", [384, 160], BF16, isOutput=False)
    w0 = nc.declare_dram_parameter("w0", [384, 128], BF16, isOutput=False)
    w1 = nc.declare_dram_parameter("w1", [128, 128], BF16, isOutput=False)
    w2 = nc.declare_dram_parameter("w2", [128, 128], BF16, isOutput=False)
    wo = nc.declare_dram_parameter("wo", [128, 128], BF16, isOutput=False)
    b0 = nc.declare_dram_parameter("b0", [128], F32, isOutput=False)
    b1 = nc.declare_dram_parameter("b1", [128], F32, isOutput=False)
    b2 = nc.declare_dram_parameter("b2", [128], F32, isOutput=False)
    st4 = nc.declare_dram_parameter("st4", [128, M], BF16, isOutput=False)
    uc = nc.declare_dram_parameter("uc", [128, 512], BF16, isOutput=False)
    b4w = nc.declare_dram_parameter("b4w", [128, 128], F32, isOutput=False)
    out = nc.declare_dram_parameter("out", [BPC, M], F32, isOutput=True)

    CW = [512] * 15 + [MREM]

    with tile.TileContext(nc) as tc:
        with (
            tc.tile_pool(name="persist", bufs=1) as persist,
            tc.tile_pool(name="mlp_sb", bufs=2) as mlp_sb,
            tc.tile_pool(name="zs_sb", bufs=33) as zs_sb,
            tc.tile_pool(name="soft_sb", bufs=2) as soft_sb,
        ):
            # ---- constant / weight loads ----
            # MLP-critical weights on the SP queue; everything else on gpsimd
            # (SWDGE) so its issue cost stays off the queues the MLP's latency
            # chain runs through.
            xTt = persist.tile([128, 3, 160], BF16, tag="xTt")
            nc.sync.dma_start(out=xTt[:], in_=xT[:].rearrange("(c p) f -> p c f", p=128))
            w0t = persist.tile([128, 3, 128], BF16, tag="w0t")
            nc.sync.dma_start(out=w0t[:], in_=w0[:].rearrange("(c p) f -> p c f", p=128))
            w1t = persist.tile([128, 128], BF16, tag="w1t")
            nc.sync.dma_start(out=w1t[:], in_=w1[:])
            w2t = persist.tile([128, 128], BF16, tag="w2t")
            nc.sync.dma_start(out=w2t[:], in_=w2[:])
            wot = persist.tile([128, 128], BF16, tag="wot")
            nc.sync.dma_start(out=wot[:], in_=wo[:])
            b0t = persist.tile([128, 1], F32, tag="b0t")
            nc.gpsimd.dma_start(out=b0t[:], in_=b0[:].rearrange("(p o) -> p o", o=1))
            b1t = persist.tile([128, 1], F32, tag="b1t")
            nc.gpsimd.dma_start(out=b1t[:], in_=b1[:].rearrange("(p o) -> p o", o=1))
            b2t = persist.tile([128, 1], F32, tag="b2t")
            nc.gpsimd.dma_start(out=b2t[:], in_=b2[:].rearrange("(p o) -> p o", o=1))
            st4t = persist.tile([128, M], BF16, tag="st4t")
            for lo, hi in ((0, 2048), (2048, 4096), (4096, 6144), (6144, M)):
                nc.gpsimd.dma_start(out=st4t[:, lo:hi], in_=st4[:, lo:hi])
            uct = persist.tile([128, 512], BF16, tag="uct")
            nc.gpsimd.dma_start(out=uct[:], in_=uc[:])
            b4wt = persist.tile([128, 128], F32, tag="b4wt")
            nc.gpsimd.dma_start(out=b4wt[:], in_=b4w[:])

            # ---- ACT spline-table preload: tiny ops on a memset tile force
            # the (single) table set to load while input DMAs are in flight.
            pre = mlp_sb.tile([128, 1], F32, tag="pre", name="pre")
            nc.vector.memset(pre[:], 0.25)
            pre2 = mlp_sb.tile([128, 1], F32, tag="pre2", name="pre2")
            nc.scalar.activation(out=pre2[:], in_=pre[:], func=AF.Prelu,
                                 bias=0.0, scale=1.0, alpha=NEG_SLOPE)
            nc.scalar.activation(out=pre2[:], in_=pre[:], func=AF.Tanh)
            nc.scalar.activation(out=pre2[:], in_=pre[:], func=AF.Exp)
            nc.scalar.copy(out=pre2[:], in_=pre[:])

            # ---- MLP (own psum pool, released before the z stream) ----
            rows = []
            with tc.tile_pool(name="mlp_ps", bufs=2, space="PSUM") as mlp_ps:
                h1ps = mlp_ps.tile([128, 160], F32, tag="mp", name="h1ps")
                for k in range(3):
                    nc.tensor.matmul(h1ps[:], w0t[:, k, :], xTt[:, k, :],
                                     start=(k == 0), stop=(k == 2))
                h1 = mlp_sb.tile([128, 160], BF16, tag="h1")
                nc.scalar.activation(out=h1[:], in_=h1ps[:], func=AF.Prelu,
                                     bias=b0t[:], scale=1.0, alpha=NEG_SLOPE)
                h2ps = mlp_ps.tile([128, 160], F32, tag="mp", name="h2ps")
                nc.tensor.matmul(h2ps[:], w1t[:], h1[:], start=True, stop=True)
                h2 = mlp_sb.tile([128, 160], BF16, tag="h2")
                nc.scalar.activation(out=h2[:], in_=h2ps[:], func=AF.Prelu,
                                     bias=b1t[:], scale=1.0, alpha=NEG_SLOPE)
                h3ps = mlp_ps.tile([128, 160], F32, tag="mp", name="h3ps")
                nc.tensor.matmul(h3ps[:], w2t[:], h2[:], start=True, stop=True)
                # h3 written 3x into [128, 8, 60] (per-batch contiguous
                # replicas): the [128, 60] stationary slice h3r[:, b, :] then
                # emits all three replicas of a batch in ONE rows-matmul.
                h3r = mlp_sb.tile([128, 8, 60], BF16, tag="h3r", name="h3r")
                for rep in range(3):
                    dst = h3r[:, :, 20 * rep:20 * rep + 20]
                    srcv = h3ps[:].rearrange("p (b n) -> p b n", n=20)
                    if rep == 2:
                        nc.vector.tensor_scalar_add(out=dst, in0=srcv,
                                                    scalar1=b2t[:])
                    else:
                        nc.scalar.activation(out=dst, in_=srcv,
                                             func=AF.Identity, bias=b2t[:],
                                             scale=1.0)
                # rows[(half,p)]: partitions 64s+(20*rep+n) = (h[4h+2p+s] @
                # Wo)[n, :] for s in {0,1}, rep in {0,1,2}; K=64 stationaries
                # for the z S-matmuls keep PE row occupancy ~94% so the clock
                # stays high through the stream. Half 1 is built FIRST: the z
                # stream's first PSUM slot reuses the MLP pool's banks, so the
                # LAST rows-copies gate the z start - make those half 0's
                # (which chunk 0 needs anyway).
                rows = [None] * 4
                for half in (1, 0):
                    for p in range(2):
                        rps = mlp_ps.tile([128, 128], F32, tag="mp",
                                          name=f"rps{half}_{p}")
                        for s_ in range(2):
                            b = half * 4 + 2 * p + s_
                            nc.tensor.matmul(
                                rps[64 * s_:64 * s_ + 60, :],
                                h3r[:, b, :], wot[:],
                                start=True, stop=True,
                                tile_position=(0, 64 * s_))
                        rt = persist.tile([128, 128], BF16,
                                          tag=f"rows{half}_{p}",
                                          name=f"rows{half}_{p}")
                        # pad rows 60-63/124-127 must be exactly 0 (not stale
                        # PSUM bits): NaN x 0 would poison the K=64 matmul.
                        # One copy each on ScalarE/VectorE so they overlap.
                        nc.vector.memset(rt[:], 0.0)
                        nc.scalar.copy(out=rt[0:60, :], in_=rps[0:60, :])
                        nc.vector.tensor_scalar_add(out=rt[64:124, :],
                                                    in0=rps[64:124, :],
                                                    scalar1=0.0)
                        rows[2 * half + p] = rt

            # ---- z stream (all 8 PSUM banks: 4-slot ring, no coupling
            # between the PE and the evac engines), then a separate u phase
            # at the warm clock with the softmax staged inside it.
            # greedy evac-engine balancing (measured sustained ns/op)
            eng_t = {"a": 0.0, "d": 0.0}

            def pick(nelem):
                ca = nelem * 0.833 + 180.0
                cd = nelem * 0.917 + 190.0
                if eng_t["a"] + ca <= eng_t["d"] + cd:
                    eng_t["a"] += ca
                    return "a"
                eng_t["d"] += cd
                return "d"

            zsts = {}
            with tc.tile_pool(name="zq_ps", bufs=4, space="PSUM") as zq_ps:

                def emit_chunk(half, c):
                    w = CW[c]
                    zst = zs_sb.tile([128, 4, 512], BF16, tag="zs",
                                     name=f"zst{half}_{c}")
                    for pr in range(2):
                        zps = zq_ps.tile([128, 2, 512], F32, tag="zq",
                                         name=f"zq{half}_{c}_{pr}")
                        for s_ in range(2):
                            nc.tensor.matmul(
                                zps[:, s_, 0:w],
                                rows[2 * half + pr][64 * s_:64 * s_ + 64, :],
                                st4t[64 * s_:64 * s_ + 64,
                                     512 * c:512 * c + w],
                                start=True, stop=True,
                                tile_position=(64 * s_, 0))
                        e = pick(2 * w)
                        src = zps[:, 0:2, 0:w]
                        dst = zst[:, 2 * pr:2 * pr + 2, 0:w]
                        if e == "a":
                            nc.scalar.activation(out=dst, in_=src, func=AF.Relu)
                        else:
                            nc.vector.tensor_scalar_max(out=dst, in0=src,
                                                        scalar1=0.0)
                    zsts[(half, c)] = zst

                for hc in [(h, c) for h in range(2) for c in range(NCHUNK)]:
                    emit_chunk(*hc)

            with tc.tile_pool(name="score_ps", bufs=1, space="PSUM") as score_ps:
                scores = []
                for half in range(2):
                    sct = score_ps.tile([128, 512], F32, tag=f"sc{half}",
                                        name=f"sc{half}")
                    scores.append(sct)

                def emit_u(half, c):
                    w = CW[c]
                    zst = zsts.pop((half, c))
                    for g in range(4):
                        nc.tensor.matmul(
                            scores[half][32 * g:32 * g + 32, 0:w],
                            uct[:, 32 * c:32 * c + 32], zst[:, g, 0:w],
                            start=(c == 0), stop=(c == NCHUNK - 1),
                            tile_position=(0, 32 * g), skip_group_check=True)

                def softmax_stages(half):
                    st = {}

                    def s1():  # tanh
                        st["t"] = soft_sb.tile([128, 512], F32, tag=f"t{half}", name=f"t{half}")
                        nc.scalar.activation(out=st["t"][:], in_=scores[half][:],
                                             func=AF.Tanh)
                        eng_t["a"] += 800.0

                    def s2():  # exp with row-sum accumulator
                        st["e"] = soft_sb.tile([128, 512], F32, tag=f"e{half}", name=f"e{half}")
                        st["rs"] = soft_sb.tile([128, 1], F32, tag=f"rs{half}", name=f"rs{half}")
                        nc.scalar.activation(out=st["e"][:], in_=st["t"][:],
                                             func=AF.Exp, scale=10.0,
                                             accum_out=st["rs"][:])
                        eng_t["a"] += 1000.0

                    def s3():  # denominator matmul (sum+pad-correct+broadcast)
                        st["rbc"] = score_ps.tile([128, 512], F32,
                                                  tag=f"sc{half}",
                                                  name=f"rbc{half}")
                        nc.tensor.matmul(st["rbc"][:, 0:1], b4wt[:],
                                         st["rs"][:], start=True, stop=True,
                                         skip_group_check=True)
                        st["rec"] = soft_sb.tile([128, 1], F32, tag=f"rec{half}", name=f"rec{half}")
                        nc.vector.reciprocal(out=st["rec"][:],
                                             in_=st["rbc"][:, 0:1])
                        eng_t["d"] += 260.0

                    def s4():  # scale + output DMAs
                        probs = soft_sb.tile([128, 512], F32, tag=f"pr{half}", name=f"pr{half}")
                        nc.vector.tensor_scalar_mul(out=probs[:], in0=st["e"][:],
                                                    scalar1=st["rec"][:])
                        eng_t["d"] += 760.0
                        # per-batch main blocks: a DMA source AP may carry only
                        # ONE partition-dim pair, so the four 15-row blocks
                        # cannot be fused into a single (g,r) pattern.
                        for g in range(4):
                            b = 4 * half + g
                            eng = nc.sync if g % 2 == 0 else nc.gpsimd
                            eng.dma_start(
                                out=out[b, 0:15 * 512]
                                    .rearrange("(r j) -> r j", j=512),
                                in_=probs[32 * g:32 * g + 15, :])
                        # remainders consolidated: stride-32 partition dim
                        # (single level) x 72 cols -> 4 DRAM rows.
                        nc.gpsimd.dma_start(
                            out=out[4 * half:4 * half + 4, 15 * 512:M]
                                .rearrange("b (x j) -> b x j", x=1),
                            in_=probs[:].rearrange("(g r) j -> g r j", g=4)
                                [:, 15:16, 0:MREM])

                    return [s1, s2, s3, s4]

                for c in range(NCHUNK):
                    emit_u(0, c)
                post = softmax_stages(0)
                for c in range(NCHUNK):
                    emit_u(1, c)
                    if post:
                        post.pop(0)()
                for s in post:
                    s()
                for s in softmax_stages(1):
                    s()

    nc.compile()
    return nc


_NC_CACHE = None


def _get_nc():
    global _NC_CACHE
    if _NC_CACHE is None:
        _NC_CACHE = _build_nc()
    return _NC_CACHE


def _host_constants(W0, b0, W1, b1, W2, b2, Wo, u):
    # binary S^T replicated at the four 32-row offsets; narrow chunk 15
    combs = list(combinations(range(N), KC))
    rows_idx = np.zeros((M, KC), np.int64)
    for i, cmb in enumerate(combs[:M]):
        rows_idx[i] = cmb
    stT = np.zeros((N, M), np.float32)
    stT[rows_idx.T, np.arange(M)[None, :].repeat(KC, 0)] = 1.0
    # K=64 layout: two 64-row groups, each holding 3 scaled replicas of S^T
    # (scales 0.375+0.375+0.25 = 1 exactly, each exact in bf16) + 4 zero rows.
    st4 = np.zeros((128, M), np.float32)
    for s in range(2):
        for rep, sc in enumerate((0.375, 0.375, 0.25)):
            st4[64 * s + 20 * rep:64 * s + 20 * rep + N, :] = sc * stT
    # one-hot u/5 columns: uc[d, 32c + r] = u[d]/5 if r == c < 16
    ucm = np.zeros((128, 512), np.float32)
    v = (u.astype(np.float64) / KC).astype(np.float32)
    for c in range(NCHUNK):
        ucm[:, 32 * c + c] = v
    # fused denominator weight: rbc[p] = sum_k b4w[k,p] * rs[k] gives, for
    # every partition p of group g, (sum_{c<16} rs[32g+c]) - PAD_ONES
    # (row 32g+16's rs is exactly 512, so -PAD_ONES/512 * rs[32g+16] = -440).
    b4w = np.zeros((128, 128), np.float32)
    for g in range(4):
        b4w[32 * g:32 * g + NCHUNK, 32 * g:32 * g + 32] = 1.0
        b4w[32 * g + NCHUNK, 32 * g:32 * g + 32] = -float(PAD_ONES) / 512.0
    w0p = np.zeros((384, 128), np.float32)
    w0p[:FEAT] = W0
    return {
        "w0": w0p.astype(ml_dtypes.bfloat16),
        "w1": np.asarray(W1, np.float32).astype(ml_dtypes.bfloat16),
        "w2": np.asarray(W2, np.float32).astype(ml_dtypes.bfloat16),
        "wo": np.asarray(Wo, np.float32).astype(ml_dtypes.bfloat16),
        "b0": np.ascontiguousarray(b0, np.float32),
        "b1": np.ascontiguousarray(b1, np.float32),
        "b2": np.ascontiguousarray(b2, np.float32),
        "st4": st4.astype(ml_dtypes.bfloat16),
        "uc": ucm.astype(ml_dtypes.bfloat16),
        "b4w": b4w,
    }


def kernel(batch, candidate_feature, W0, b0, W1, b1, W2, b2, Wo, u):
    cf = np.ascontiguousarray(np.asarray(candidate_feature), dtype=np.float32)
    consts = _host_constants(np.asarray(W0), np.asarray(b0), np.asarray(W1),
                             np.asarray(b1), np.asarray(W2), np.asarray(b2),
                             np.asarray(Wo), np.asarray(u))
    in_maps = []
    for core in range(N_CORES):
        xc = cf[core * BPC:(core + 1) * BPC]            # [8, 20, 320]
        xTp = np.zeros((384, 160), np.float32)
        xTp[:FEAT] = xc.reshape(BPC * N, FEAT).T        # col = b*20 + n
        in_maps.append({"xT": xTp.astype(ml_dtypes.bfloat16), **consts})
    nc = _get_nc()
    res = run_bass_kernel_spmd(nc, in_maps, core_ids=list(range(N_CORES)))
    return np.concatenate([res.results[i]["out"] for i in range(N_CORES)], axis=0)
